# revision 1
# baseline (speedup 1.0000x reference)
"""Multi-head attention (B=4, S=2048, d_model=1024, H=16) on 8 TRN2 NeuronCores.

Sharding: tensor-parallel over heads x data-parallel over batch.
Core c handles batch b=c//2 and head group g=c%2 (8 heads = 512 of the
1024 d_model columns of W_Q/W_K/W_V, and 512 rows of W_O). Each core
produces a partial output Y_partial[b] = O_g @ W_O[g-rows, :]; the host
sums the two partials per batch.

Device-side dataflow per core (all matmul operands fp16, accum fp32):
  - inputs arrive pre-transposed: X^T in [dmodel, token] layout
  - k^T, q^T = W^T X^T         (lhsT = W chunk, rhs = X^T chunk)
  - v = X @ W_V   in [token, head-dim] layout, with a ones column
  - per head h, per 128-ktok block: scores^T = k^T.T q^T  -> PSUM
    exp (scale=1/8 fused into the activation)             -> P^T fp16
    out^T_ext += [v_h | 1].T @ P^T   (row 64 = softmax denominator)
  - out^T / denominator -> O^T; Y_partial = O @ W_O slice -> DRAM
"""

import numpy as np

B = 4
S = 2048
D = 1024
H = 16
DK = 64
NCORES = 8
HPC = 8          # heads per core
GCOLS = 512      # d_model columns per head group
QB = 512         # q-token block (PSUM bank free dim)
NQB = S // QB    # 4
NKB = S // 128   # 16 k-token blocks
NC_CHUNKS = D // 128  # 8 contraction chunks

_prog_cache = {}


def build_program(reps=1, phases=("proj", "attn", "out"), skips=()):
    """Build + compile the SPMD program. Cached per (reps, phases, skips)."""
    key = (reps, tuple(phases), tuple(skips))
    if key in _prog_cache:
        return _prog_cache[key]

    import concourse.bacc as bacc
    import concourse.mybir as mybir
    from concourse.tile import TileContext

    f16 = mybir.dt.float16
    f32 = mybir.dt.float32
    EXP = mybir.ActivationFunctionType.Exp

    nc = bacc.Bacc("TRN2", target_bir_lowering=False, debug=False,
                   num_devices=NCORES)

    # DRAM parameters (per-core shards, pre-laid-out on host)
    qt_d = nc.dram_tensor("qt", [128, NC_CHUNKS, S], f16, kind="ExternalInput").ap()
    kt_d = nc.dram_tensor("kt", [128, NC_CHUNKS, S], f16, kind="ExternalInput").ap()
    vt_d = nc.dram_tensor("vt", [128, NC_CHUNKS, S], f16, kind="ExternalInput").ap()
    wq_d = nc.dram_tensor("wq", [128, NC_CHUNKS, GCOLS], f16, kind="ExternalInput").ap()
    wk_d = nc.dram_tensor("wk", [128, NC_CHUNKS, GCOLS], f16, kind="ExternalInput").ap()
    wv_d = nc.dram_tensor("wv", [128, NC_CHUNKS, GCOLS], f16, kind="ExternalInput").ap()
    wo_d = nc.dram_tensor("wo", [128, 4, D], f16, kind="ExternalInput").ap()
    yp_d = nc.dram_tensor("yp", [S, D], f32, kind="ExternalOutput").ap()

    with TileContext(nc) as tc:
        with tc.tile_pool(name="weights", bufs=1) as wpool, \
             tc.tile_pool(name="xt", bufs=2) as xtpool, \
             tc.tile_pool(name="proj", bufs=1) as projpool, \
             tc.tile_pool(name="work", bufs=2) as workpool, \
             tc.tile_pool(name="psum", bufs=1, space="PSUM") as psp:

          for rep in range(reps):
            # ---- load weights (resident) ----
            wq_sb = wpool.tile([128, NC_CHUNKS, GCOLS], f16, name="wq_sb", tag="wq")
            wk_sb = wpool.tile([128, NC_CHUNKS, GCOLS], f16, name="wk_sb", tag="wk")
            wv_sb = wpool.tile([128, NC_CHUNKS, GCOLS], f16, name="wv_sb", tag="wv")
            wo_sb = wpool.tile([128, 4, D], f16, name="wo_sb", tag="wo")
            nc.sync.dma_start(out=wv_sb[:], in_=wv_d[:])

            # ---- projection outputs (resident) ----
            # kT/qT: [dk-on-partitions, token]; chunk j holds head 2j on
            # partitions 0:64 and head 2j+1 on 64:128
            kT_sb = projpool.tile([128, 4, S], f16, name="kT_sb", tag="kT")
            qT_sb = projpool.tile([128, 4, S], f16, name="qT_sb", tag="qT")
            # v: [token-on-partitions, head, dim(+ones col at 64)]
            v_sb = projpool.tile([128, NKB, HPC, 66], f16, name="v_sb", tag="v")
            oT_sb = projpool.tile([128, 4, S], f16, name="oT_sb", tag="oT")

            if "proj" not in phases:
                nc.vector.memset(kT_sb[:], 0.01)
                nc.vector.memset(qT_sb[:], 0.01)
                nc.vector.memset(v_sb[:], 0.01)
            if "attn" not in phases:
                nc.vector.memset(oT_sb[:], 0.01)

            def proj_half(w_sb, xt_sb, dst, m, n, half, ps_holder):
                # half a projection tile: 4 of 8 contraction chunks
                if half == 0:
                    ps_holder[0] = psp.tile([128, QB], f32, name="proj_ps",
                                            tag="pps", bufs=2)
                ps = ps_holder[0]
                for c in range(4 * half, 4 * half + 4):
                    nc.tensor.matmul(
                        ps[:],
                        w_sb[:, c, m * 128:(m + 1) * 128],
                        xt_sb[:, c, n * QB:(n + 1) * QB],
                        start=(c == 0), stop=(c == NC_CHUNKS - 1))
                if half == 1:
                    nc.vector.tensor_copy(dst[:, m, n * QB:(n + 1) * QB],
                                          ps[:])

            def proj_unit(w_sb, xt_sb, dst, m, n):
                holder = [None]
                proj_half(w_sb, xt_sb, dst, m, n, 0, holder)
                proj_half(w_sb, xt_sb, dst, m, n, 1, holder)

            def attention_pair(j, pending):
                h0, h1 = 2 * j, 2 * j + 1
                # staging for unnormalized attention outputs of this pair:
                # h0 on partitions 0:64, h1 on 64:128; free dim = (qb, tok)
                unnorm0 = workpool.tile([64, NQB, QB], f16, name="unnorm0",
                                        tag="unnorm0", bufs=1)
                unnorm1 = workpool.tile([64, NQB, QB], f16, name="unnorm1",
                                        tag="unnorm1", bufs=1)
                it = 0
                deferred = [None]
                for qb in range(NQB):
                    out0 = psp.tile([128, QB], f32, name="out0", tag="out0",
                                    bufs=1)
                    out1 = psp.tile([128, QB], f32, name="out1", tag="out1",
                                    bufs=1)
                    for kb in range(NKB):
                        sb2 = psp.tile([128, 2, QB], f32, name="sb2",
                                       tag="sbig", bufs=2)
                        # row-packed score pair: head h0 on PE rows 0:64,
                        # head h1 on rows 64:128 (concurrent in the array)
                        nc.tensor.matmul(
                            sb2[:, 0, :],
                            kT_sb[0:64, j, kb * 128:(kb + 1) * 128],
                            qT_sb[0:64, j, qb * QB:(qb + 1) * QB],
                            start=True, stop=True)
                        nc.tensor.matmul(
                            sb2[:, 1, :],
                            kT_sb[64:128, j, kb * 128:(kb + 1) * 128],
                            qT_sb[64:128, j, qb * QB:(qb + 1) * QB],
                            start=True, stop=True)
                        pT = workpool.tile([128, 2, QB], f16, name="pT",
                                           tag="pT", bufs=3)
                        nc.scalar.activation(
                            pT[:].rearrange("p a b -> p (a b)"),
                            sb2[:].rearrange("p a b -> p (a b)"),
                            EXP, scale=0.125)
                        nc.tensor.matmul(
                            out0[0:65, :], v_sb[:, kb, h0, 0:65], pT[:, 0, :],
                            start=(kb == 0), stop=(kb == NKB - 1))
                        nc.tensor.matmul(
                            out1[0:65, :], v_sb[:, kb, h1, 0:65], pT[:, 1, :],
                            start=(kb == 0), stop=(kb == NKB - 1))
                        # keep PE dense: slot next chunk's projection work in
                        it += 1
                        if pending and (it % 4 == 0 or
                                        (len(pending) > 16 and it % 2 == 0)):
                            pending.pop(0)()
                    # stage to SBUF fast (frees the PSUM accumulators);
                    # the reciprocal+normalize for this qb is emitted one qb
                    # later so these copies never queue behind it on DVE
                    db0 = workpool.tile([1, QB], f32, name="db0", tag="db0",
                                        bufs=2)
                    db1 = workpool.tile([1, QB], f32, name="db1", tag="db1",
                                        bufs=2)
                    nc.vector.tensor_copy(unnorm0[:, qb, :], out0[0:64, :])
                    nc.vector.tensor_copy(db0[:], out0[64:65, :])
                    nc.vector.tensor_copy(unnorm1[:, qb, :], out1[0:64, :])
                    nc.vector.tensor_copy(db1[:], out1[64:65, :])

                    def _normalize(qb=qb, db0=db0, db1=db1):
                        rcp0 = workpool.tile([1, QB], f32, name="rcp0",
                                             tag="rcp0", bufs=1)
                        rcp1 = workpool.tile([1, QB], f32, name="rcp1",
                                             tag="rcp1", bufs=1)
                        nc.vector.reciprocal(rcp0[:], db0[:])
                        nc.vector.reciprocal(rcp1[:], db1[:])
                        rcph = workpool.tile([1, 2, QB], f16, name="rcph",
                                             tag="rcph", bufs=1)
                        nc.vector.tensor_copy(rcph[:, 0, :], rcp0[:])
                        nc.vector.tensor_copy(rcph[:, 1, :], rcp1[:])
                        rbc = workpool.tile([64, 2, QB], f16, name="rbc",
                                            tag="rbc", bufs=1)
                        nc.gpsimd.partition_broadcast(rbc[:, 0, :],
                                                      rcph[0:1, 0, :])
                        nc.gpsimd.partition_broadcast(rbc[:, 1, :],
                                                      rcph[0:1, 1, :])
                        nc.vector.tensor_mul(
                            oT_sb[0:64, j, qb * QB:(qb + 1) * QB],
                            unnorm0[:, qb, :], rbc[:, 0, :])
                        nc.vector.tensor_mul(
                            oT_sb[64:128, j, qb * QB:(qb + 1) * QB],
                            unnorm1[:, qb, :], rbc[:, 1, :])

                    if deferred[0] is not None:
                        deferred[0]()
                    deferred[0] = _normalize
                deferred[0]()
                while pending:
                    pending.pop(0)()

            if "proj" in phases:
                # V first (vt slot frees for qt right after); per-chunk DMAs
                # so projection matmuls start as soon as chunk c lands
                vt_sb = xtpool.tile([128, NC_CHUNKS, S], f16, name="xt_sb",
                                    tag="xt")
                for c in range(NC_CHUNKS):
                    nc.sync.dma_start(out=vt_sb[:, c, :], in_=vt_d[:, c, :])
                kt_sb = xtpool.tile([128, NC_CHUNKS, S], f16, name="xt_sb",
                                    tag="xt")
                nc.sync.dma_start(out=wk_sb[:], in_=wk_d[:])
                for c in range(NC_CHUNKS):
                    nc.sync.dma_start(out=kt_sb[:, c, :], in_=kt_d[:, c, :])
                for kb in range(NKB):
                    nc.vector.memset(v_sb[:, kb, :, :], 1.0)
                for kb in range(NKB):
                    ps = psp.tile([128, GCOLS], f32, name="vproj_ps",
                                  tag="pps", bufs=2)
                    for c in range(NC_CHUNKS):
                        nc.tensor.matmul(
                            ps[:],
                            vt_sb[:, c, kb * 128:(kb + 1) * 128],
                            wv_sb[:, c, :],
                            start=(c == 0), stop=(c == NC_CHUNKS - 1))
                    nc.vector.tensor_copy(
                        v_sb[:, kb, :, 0:64],
                        ps[:].rearrange("p (h d) -> p h d", h=HPC))
                proj_unit(wk_sb, kt_sb, kT_sb, 0, 0)
                qt_sb = xtpool.tile([128, NC_CHUNKS, S], f16, name="xt_sb",
                                    tag="xt")
                nc.sync.dma_start(out=wq_sb[:], in_=wq_d[:])
                for c in range(NC_CHUNKS):
                    nc.sync.dma_start(out=qt_sb[:, c, :], in_=qt_d[:, c, :])
                nc.sync.dma_start(out=wo_sb[:], in_=wo_d[:])
                proj_unit(wk_sb, kt_sb, kT_sb, 0, 1)
                proj_unit(wq_sb, qt_sb, qT_sb, 0, 0)
                head_pending = []
                for n_, w_, x_, d_ in ((2, wk_sb, kt_sb, kT_sb),
                                       (3, wk_sb, kt_sb, kT_sb),
                                       (1, wq_sb, qt_sb, qT_sb),
                                       (2, wq_sb, qt_sb, qT_sb),
                                       (3, wq_sb, qt_sb, qT_sb)):
                    hold = [None]
                    for half in range(2):
                        head_pending.append(
                            lambda w_=w_, x_=x_, d_=d_, n=n_, half=half,
                            hold=hold: proj_half(w_, x_, d_, 0, n, half, hold))
                for j in range(4):
                    pending = list(head_pending) if j == 0 else []
                    head_pending = []
                    if j < 3:
                        for w_, x_, d_ in ((wk_sb, kt_sb, kT_sb),
                                           (wq_sb, qt_sb, qT_sb)):
                            for n in range(NQB):
                                hold = [None]
                                for half in range(2):
                                    pending.append(
                                        lambda w_=w_, x_=x_, d_=d_, m=j + 1,
                                        n=n, half=half, hold=hold: proj_half(
                                            w_, x_, d_, m, n, half, hold))
                    if "attn" in phases:
                        attention_pair(j, pending)
                    else:
                        while pending:
                            pending.pop(0)()
            elif "attn" in phases:
                for j in range(4):
                    attention_pair(j, [])

            # ---- output projection ----
            if "out" in phases:
                for t in range(NKB):
                    y_sb = workpool.tile([128, D], f32, name="y_sb",
                                         tag="y", bufs=2)
                    for n2 in range(2):
                        ps = psp.tile([128, QB], f32, name="y_ps", tag="pps",
                                      bufs=2)
                        for c2 in range(4):
                            nc.tensor.matmul(
                                ps[:],
                                oT_sb[:, c2, t * 128:(t + 1) * 128],
                                wo_sb[:, c2, n2 * QB:(n2 + 1) * QB],
                                start=(c2 == 0), stop=(c2 == 3))
                        nc.vector.tensor_copy(y_sb[:, n2 * QB:(n2 + 1) * QB],
                                              ps[:])
                    nc.sync.dma_start(out=yp_d[t * 128:(t + 1) * 128, :],
                                      in_=y_sb[:])

    nc.compile()
    _prog_cache[key] = nc
    return nc


def _chunk_pT(x):
    """[S, D] -> [128, D//128, S] fp16 (X^T chunked: out[p, c, t] = x[t, 128c+p])."""
    a = np.ascontiguousarray(x.reshape(S, NC_CHUNKS, 128).transpose(2, 1, 0))
    return a


def _chunk_w(w):
    """[D, GCOLS] -> [128, 8, GCOLS]: out[p, c, m] = w[128c+p, m]."""
    return np.ascontiguousarray(
        w.reshape(NC_CHUNKS, 128, w.shape[1]).transpose(1, 0, 2))


def prepare_in_maps(Q, K, V, W_Q, W_K, W_V, W_O):
    f16 = np.float16
    qt = [_chunk_pT(Q[b].astype(f16)) for b in range(B)]
    kt = [_chunk_pT(K[b].astype(f16)) for b in range(B)]
    vt = [_chunk_pT(V[b].astype(f16)) for b in range(B)]
    wq = [_chunk_w(W_Q[:, g * GCOLS:(g + 1) * GCOLS].astype(f16)) for g in range(2)]
    wk = [_chunk_w(W_K[:, g * GCOLS:(g + 1) * GCOLS].astype(f16)) for g in range(2)]
    wv = [_chunk_w(W_V[:, g * GCOLS:(g + 1) * GCOLS].astype(f16)) for g in range(2)]
    # wo rows for group g, chunked: [128, 4, D]
    wo = [np.ascontiguousarray(
        W_O[g * GCOLS:(g + 1) * GCOLS, :].astype(f16)
        .reshape(4, 128, D).transpose(1, 0, 2)) for g in range(2)]
    in_maps = []
    for c in range(NCORES):
        b, g = c // 2, c % 2
        in_maps.append({
            "qt": qt[b], "kt": kt[b], "vt": vt[b],
            "wq": wq[g], "wk": wk[g], "wv": wv[g], "wo": wo[g],
        })
    return in_maps


def execute(nc, in_maps):
    from concourse.bass_utils import run_bass_kernel_spmd
    res = run_bass_kernel_spmd(nc, in_maps, list(range(NCORES)))
    return res


def _numpy_fallback(Q, K, V, mask, W_Q, W_K, W_V, W_O):
    import math
    B_, S1, _ = Q.shape
    q = (Q.reshape(-1, D) @ W_Q).reshape(B_, S1, H, DK).transpose(0, 2, 1, 3)
    k = (K.reshape(-1, D) @ W_K).reshape(B_, S1, H, DK).transpose(0, 2, 1, 3)
    v = (V.reshape(-1, D) @ W_V).reshape(B_, S1, H, DK).transpose(0, 2, 1, 3)
    out = np.empty((B_, H, S1, DK), np.float32)
    for b in range(B_):
        for h in range(H):
            s = (q[b, h] @ k[b, h].T) / math.sqrt(DK)
            s = np.where(mask[b] == 0, np.float32(-1e9), s)
            s = s - s.max(axis=-1, keepdims=True)
            e = np.exp(s)
            p = e / e.sum(axis=-1, keepdims=True)
            out[b, h] = p @ v[b, h]
    o = out.transpose(0, 2, 1, 3).reshape(B_, S1, D)
    return (o.reshape(-1, D) @ W_O).reshape(B_, S1, D).astype(np.float32)


def kernel(Q, K, V, mask, W_Q, W_K, W_V, W_O):
    Q = np.asarray(Q); K = np.asarray(K); V = np.asarray(V)
    mask = np.asarray(mask)
    W_Q = np.asarray(W_Q); W_K = np.asarray(W_K)
    W_V = np.asarray(W_V); W_O = np.asarray(W_O)
    if (mask == 0).any():
        # spec guarantees an all-ones mask; this path is correctness insurance
        return _numpy_fallback(Q, K, V, mask, W_Q, W_K, W_V, W_O)
    nc = build_program()
    in_maps = prepare_in_maps(Q, K, V, W_Q, W_K, W_V, W_O)
    res = execute(nc, in_maps)
    out = np.empty((B, S, D), np.float32)
    for b in range(B):
        out[b] = res.results[2 * b]["yp"] + res.results[2 * b + 1]["yp"]
    return out

